# revision 48
# baseline (speedup 1.0000x reference)
"""3-layer GCN forward on 8 TRN2 NeuronCores (Bass/Tile), v3.

Math: per layer, out = dinv * ((A+I) @ T) @ W + b with T = dinv*h,
dinv = 1/sqrt(deg+1); leaky_relu(0.2) between layers.

v3 changes vs v2:
- Self-loop tokens eliminated: the +T_own term is an identity-matmul into
  the same PSUM accumulation, reading the locally kept own-block tile.
- AllGather replaced by remote_dma_broadcast (SBUF->SBUF to all 8 cores,
  Switch on partition id selects the sender's slice) + one SBUF->DRAM
  copy that rebuilds the gather table; a data-free broadcast barriers
  table_sb reuse.
- Table row order is (p, c, t): node at core c, tile t, slot p lives at
  row (p*8+c)*NTILES+t, so the staging SBUF tile [128, 8*NTILES*64] is
  DRAM-linear and the table copy runs at full descriptor size.
- Gathers are issued per super-tile (SUP tiles x parity) instead of per
  tile: 14 vs 98 SWDGE desc-gen fixed overheads per layer.
- One-hot built in "a-last" layout oh[p, c*P+a] so all tensor_tensor
  operands have stride-1 last dims (DVE 2x mode); matmul lhsT reads
  plane a with a stride-P access pattern.
- Bias rows folded into the weight matmul ([W; b], ones row in lhsT);
  dinv scaling + leaky fused into Activation-engine ops
  (leaky(dinv*x) == dinv*leaky(x) for dinv > 0).
"""
import numpy as np

NEG_SLOPE = 0.2
_TIMING = False  # strip cross-core sem waits so TimelineSim can run
_DEBUG = False


class _Cfg:
    def __init__(self, n_nodes, d_in=64, d_out=4, sup_tiles=None):
        self.W = 8
        self.N = n_nodes
        self.D = d_in
        self.DOUT = d_out
        self.BLK = (n_nodes + self.W - 1) // self.W
        self.BP = ((self.BLK + 127) // 128) * 128
        self.NT = self.W * self.BP
        self.NPAIR = self.NT // 2
        self.NTILES = self.BP // 128
        if sup_tiles is None:
            sup_tiles = 7 if self.NTILES % 7 == 0 else 1
        self.SUP = sup_tiles if self.NTILES % sup_tiles == 0 else 1
        self.NSUP = self.NTILES // self.SUP
        assert self.NPAIR <= 32768, self.NPAIR


def _balance_tiles(cfg, tok_counts):
    """Assign this core's nodes (local ids) to (tile, slot) so that
    per-(tile,parity) token counts are even. tok_counts: [nloc, 2] int.
    Returns perm[nloc] -> tile*128+slot."""
    nloc = tok_counts.shape[0]
    ntiles = cfg.NTILES
    cap = np.full(ntiles, 128, np.int64)
    load = np.zeros((ntiles, 2), np.float64)
    order = np.argsort(-(tok_counts.sum(1)))
    perm = np.zeros(nloc, np.int64)
    slots_used = np.zeros(ntiles, np.int64)
    for n in order:
        t0, t1 = tok_counts[n]
        cand = np.where(slots_used < cap)[0]
        v = np.maximum(load[cand, 0] + t0, load[cand, 1] + t1)
        best = cand[np.argmin(v)]
        perm[n] = best * 128 + slots_used[best]
        slots_used[best] += 1
        load[best, 0] += t0
        load[best, 1] += t1
    return perm


def _refine_caps(cfg, e, perm, caps, iters=4000):
    """Same-parity-tile swap refinement: push per-(tile,parity) loads
    under caps (so plane counts hit floor instead of ceil). Swapping
    nodes between tiles t, t' with t ≡ t' (mod 2) preserves every node's
    source parity ((c+t)&1), so other cores' schedules are unaffected."""
    ntiles = cfg.NTILES
    tile = (perm >> 7).copy()
    slot = (perm & 127).copy()
    loads = np.zeros((ntiles, 2), np.int64)
    np.add.at(loads, (tile, 0), e[:, 0])
    np.add.at(loads, (tile, 1), e[:, 1])
    nodes_in = [np.where(tile == t)[0] for t in range(ntiles)]

    def overs(ld):
        return np.maximum(ld - caps, 0).sum()

    for _ in range(iters):
        over = loads - caps
        om = over.max(1)
        w = int(np.argmax(om))
        if om[w] <= 0:
            break
        q = int(np.argmax(over[w]))
        par_tiles = np.arange(w & 1, ntiles, 2)
        par_tiles = par_tiles[par_tiles != w]
        # most headroom in q first
        t2s = par_tiles[np.argsort(over[par_tiles, q])][:4]
        un = nodes_in[w]
        best = None
        for t2 in t2s:
            vn = nodes_in[t2]
            # delta for swapping u (from w) with v (from t2):
            du = e[un]                     # [nu, 2]
            dv = e[vn]                     # [nv, 2]
            base = (max(over[w, 0], 0) + max(over[w, 1], 0)
                    + max(over[t2, 0], 0) + max(over[t2, 1], 0))
            d = dv[None, :, :] - du[:, None, :]   # [nu, nv, 2] add to w
            nw = loads[w] + d
            nt = loads[t2] - d
            cost = (np.maximum(nw[:, :, 0] - caps[w, 0], 0)
                    + np.maximum(nw[:, :, 1] - caps[w, 1], 0)
                    + np.maximum(nt[:, :, 0] - caps[t2, 0], 0)
                    + np.maximum(nt[:, :, 1] - caps[t2, 1], 0))
            i, j = np.unravel_index(np.argmin(cost), cost.shape)
            gain = base - cost[i, j]
            if best is None or gain > best[0]:
                best = (gain, un[i], vn[j], t2)
        if best is None or best[0] <= 0:
            break
        _, u, v, t2 = best
        loads[w] += e[v] - e[u]
        loads[t2] += e[u] - e[v]
        tile[u], tile[v] = t2, w
        slot[u], slot[v] = slot[v], slot[u]
        nodes_in[w] = np.where(tile == w)[0]
        nodes_in[t2] = np.where(tile == t2)[0]
    return tile * 128 + slot


def _preprocess(cfg, edge_index):
    """Build shared plane schedule + per-core gidx/sid + per-core node
    permutations and dinv. Table row of node (c, t, p) is
    (p*8 + c)*NTILES + t on every core; the boundary exchange picks the
    sender's slice with a Switch on the partition id."""
    W, N, BLK, NTILES = cfg.W, cfg.N, cfg.BLK, cfg.NTILES
    SUP, NSUP = cfg.SUP, cfg.NSUP
    src = np.asarray(edge_index[0], np.int64)
    dst = np.asarray(edge_index[1], np.int64)
    deg = np.bincount(dst, minlength=N).astype(np.float64) + 1.0
    dinv = (1.0 / np.sqrt(deg)).astype(np.float32)

    node_core = np.minimum(np.arange(N) // BLK, W - 1)

    perms = []
    core_edges = []
    for c in range(W):
        lo, hi = c * BLK, min((c + 1) * BLK, N)
        m = (dst >= lo) & (dst < hi)
        s_c, d_c = src[m], dst[m] - lo
        core_edges.append((s_c, d_c))
        nloc = hi - lo
        # proxy parity first; refined with exact rows below
        tc = np.zeros((nloc, 2), np.int64)
        np.add.at(tc, (d_c, s_c % 2), 1)
        perms.append(_balance_tiles(cfg, tc))

    def rows_on(r, nodes):
        """Table row of `nodes` (same on every receiver: slice = sender)."""
        del r
        c = node_core[nodes]
        pos = np.zeros(len(nodes), np.int64)
        for c2 in range(W):
            m2 = c == c2
            if m2.any():
                pos[m2] = perms[c2][nodes[m2] - c2 * BLK]
        t = pos >> 7
        p = pos & 127
        return (p * W + c) * NTILES + t

    for _sweep in range(2):
        for r in range(W):
            s_c, d_c = core_edges[r]
            nloc = min((r + 1) * BLK, N) - r * BLK
            par = rows_on(r, s_c) & 1
            tc = np.zeros((nloc, 2), np.int64)
            np.add.at(tc, (d_c, par), 1)
            perms[r] = _balance_tiles(cfg, tc)

    # cap-targeted refinement: push every (tile,parity) bucket under
    # 8*128 tokens (designated overflow tiles get 9*128) so plane counts
    # hit the floor. Same-parity swaps keep all parities fixed, so the
    # exact counts below stay valid across cores.
    es, totals = [], []
    for r in range(W):
        s_c, d_c = core_edges[r]
        nloc = min((r + 1) * BLK, N) - r * BLK
        par = rows_on(r, s_c) & 1
        tc = np.zeros((nloc, 2), np.int64)
        np.add.at(tc, (d_c, par), 1)
        es.append(tc)
        totals.append(int(tc.sum()))
    capbase = 8 * 128
    ncap = 2 * NTILES * capbase
    OV = 0
    if max(totals) > ncap - 2 * 128:
        OV = int(np.ceil((max(totals) - (ncap - 2 * 128)) / 256.0)) + 1
        OV = min(OV, NTILES)
    caps = np.full((NTILES, 2), capbase, np.int64)
    caps[:OV] += 128
    for r in range(W):
        perms[r] = _refine_caps(cfg, es[r], perms[r], caps)

    per_core_tok = []
    for r in range(W):
        s_c, d_c = core_edges[r]
        rr = rows_on(r, s_c)
        pair, par = rr >> 1, rr & 1
        pos = perms[r][d_c]
        tile, slot = pos >> 7, pos & 127
        per_core_tok.append((pair, par, tile, slot))

    counts = np.zeros((W, NTILES, 2), np.int64)
    for c in range(W):
        _, par, tile, _ = per_core_tok[c]
        np.add.at(counts[c], (tile, par), 1)
    planes_tq = (counts.max(0) + 127) // 128  # [NTILES, 2], may be 0

    # chunks: (g, q); real planes = sum of planes_tq over tiles of g
    chunks = []  # per chunk: dict(g, q, rp, idx_base, tiles=[trel per a])
    idx_base = 0
    for g in range(NSUP):
        for q in (0, 1):
            tl = []
            for trel in range(SUP):
                t = g * SUP + trel
                tl += [trel] * int(planes_tq[t, q])
            chunks.append(dict(g=g, q=q, rp=len(tl), idx_base=idx_base,
                               tiles=tl))
            idx_base += len(tl) * 128
    ntok = idx_base
    P = max((ch["rp"] for ch in chunks), default=1)
    P = max(P, 1)
    nchunk = len(chunks)

    # per-core gidx / sid fill
    gidxs, sids = [], []
    for c in range(W):
        pair, par, tile, slot = per_core_tok[c]
        gi = np.zeros(ntok, np.int64)             # pad -> pair row 0
        sd = np.full((nchunk, P, 128), -1.0, np.float32)
        key = tile * 2 + par
        order = np.argsort(key, kind='stable')
        ks, ps, ss = key[order], pair[order], slot[order]
        bounds = np.searchsorted(ks, np.arange(NTILES * 2 + 1))
        for ci, ch in enumerate(chunks):
            g, q, rp = ch["g"], ch["q"], ch["rp"]
            base = ch["idx_base"]
            a = 0
            for trel in range(SUP):
                t = g * SUP + trel
                u, v = bounds[t * 2 + q], bounds[t * 2 + q + 1]
                cnt = v - u
                npl = int(planes_tq[t, q])
                assert cnt <= npl * 128, (c, t, q, cnt)
                for j in range(npl):
                    lo2 = u + j * 128
                    hi2 = min(u + (j + 1) * 128, v)
                    n2 = max(hi2 - lo2, 0)
                    if n2 > 0:
                        gi[base + a * 128: base + a * 128 + n2] = ps[lo2:hi2]
                        sd[ci, a, :n2] = ss[lo2:hi2]
                    a += 1
            assert a == rp
        gidxs.append(gi.astype(np.int16))
        sids.append(sd)
    sched = dict(ntok=ntok, P=P, nchunk=nchunk, chunks=chunks,
                 planes_tq=planes_tq)
    return dinv, perms, sched, gidxs, sids


def _wrap16(a):
    a = np.asarray(a, np.int16)
    assert a.size % 16 == 0
    w = np.ascontiguousarray(a.reshape(-1, 16).T)
    return np.tile(w, (8, 1))


def _build(cfg, sched):
    import concourse.bacc as bacc
    import concourse.mybir as mybir
    import concourse.tile as tile
    import concourse.masks as masks
    from concourse.bass import AP

    D, DOUT = cfg.D, cfg.DOUT
    W, BP, NPAIR, NTILES = cfg.W, cfg.BP, cfg.NPAIR, cfg.NTILES
    SUP, NSUP = cfg.SUP, cfg.NSUP
    f32, bf16, i16 = mybir.dt.float32, mybir.dt.bfloat16, mybir.dt.int16
    EQ = mybir.AluOpType.is_equal
    ACT = mybir.ActivationFunctionType
    P = sched['P']
    ntok = sched['ntok']
    chunks = sched['chunks']
    FBLK = NTILES * D          # free elems per core block (per partition)

    import contextlib
    nc = bacc.Bacc(None, target_bir_lowering=False,
                   dynamic_dma_scratch_size=65536)
    x_table = nc.dram_tensor("x_table", [NPAIR, 128], bf16,
                             kind="ExternalInput")
    x_own = nc.dram_tensor("x_own", [128, FBLK], bf16, kind="ExternalInput")
    wa = nc.dram_tensor("wa", [D + 1, D], f32, kind="ExternalInput")
    wb = nc.dram_tensor("wb", [D + 1, D], f32, kind="ExternalInput")
    wc = nc.dram_tensor("wc", [D + 1, DOUT], f32, kind="ExternalInput")
    dinv_in = nc.dram_tensor("dinv_blk", [128, NTILES], f32,
                             kind="ExternalInput")
    gidx_in = nc.dram_tensor("gidx", [128, max(ntok // 16, 1)], i16,
                             kind="ExternalInput")
    sid_in = nc.dram_tensor("sid", [128, sched['nchunk'] * P], bf16,
                            kind="ExternalInput")
    outr = nc.dram_tensor("outr", [BP, DOUT], f32, kind="ExternalOutput")
    cc_t = [nc.dram_tensor(f"cc_t{i}", [NPAIR, 128], bf16) for i in (0, 1)]

    with tile.TileContext(nc) as tc:
        with (
            tc.tile_pool(name="const", bufs=1) as cpool,
            tc.tile_pool(name="msg", bufs=2) as msgpool,
            tc.tile_pool(name="oh", bufs=2) as ohpool,
            tc.tile_pool(name="stage", bufs=2) as stpool,
            tc.tile_pool(name="psum", bufs=2, space="PSUM") as psum,
            tc.tile_pool(name="psum_e", bufs=2, space="PSUM") as psum_e,
            tc.tile_pool(name="psum_t", bufs=2, space="PSUM") as psum_t,
        ):
            gsem = [nc.alloc_semaphore(f"gsem{i}") for i in range(4)]
            gcnt = [0] * 4
            psem = nc.alloc_semaphore("psem")
            rsemA = nc.alloc_semaphore("rsemA")
            rsemB = nc.alloc_semaphore("rsemB")
            lsem = nc.alloc_semaphore("lsem")
            lsem2 = nc.alloc_semaphore("lsem2")
            bsem = nc.alloc_semaphore("bsem")
            csem = nc.alloc_semaphore("csem")
            pcnt = [0]
            ccnt = [0]

            wat = cpool.tile([D + 1, D], f32)
            wbt = cpool.tile([D + 1, D], f32)
            wct = cpool.tile([D + 1, DOUT], f32)
            dinvt = cpool.tile([128, NTILES], f32)
            d02 = cpool.tile([128, NTILES], f32)
            d08 = cpool.tile([128, NTILES], f32)
            gixt = cpool.tile([128, max(ntok // 16, 1)], i16)
            # only the first chunk's index slice loads before the first
            # gather; everything else is deferred past its desc-gen
            g0w = min(max(chunks[0]["rp"] * 8, 1), max(ntok // 16, 1))
            nc.sync.dma_start(gixt[:, 0:g0w], gidx_in[:, 0:g0w])
            sidt = cpool.tile([128, sched['nchunk'] * P], bf16)
            ident = cpool.tile([128, 128], f32)
            masks.make_identity(nc, ident[:])
            identb = cpool.tile([128, 128], bf16)
            nc.vector.tensor_copy(identb[:], ident[:])
            # iotaT[p, c*P + a] = c: tiny Pool iota row (0.2us) + DVE
            # broadcast-expand, which runs while DVE is otherwise idle --
            # a full Pool iota would block early gather desc-gen ~10us
            iotar = cpool.tile([128, 128], bf16)
            nc.gpsimd.iota(iotar[:], pattern=[[1, 128]], base=0,
                           channel_multiplier=0,
                           allow_small_or_imprecise_dtypes=True)
            iotat = cpool.tile([128, P * 128], bf16)
            ira = iotar[:]
            nc.vector.tensor_copy(
                AP(iotat[:].tensor, iotat[:].offset,
                   [list(iotat[:].ap[0]), [P, 128], [1, P]]),
                AP(ira.tensor, ira.offset,
                   [list(ira.ap[0]), [1, 128], [0, P]]))
            iota_emitted = [False]
            # own-block staging (prev layer T) + current layer output
            ccA = cpool.tile([128, FBLK], bf16)
            ccB = cpool.tile([128, FBLK], bf16)
            # broadcast assembly + final out buffers
            tabsb = cpool.tile([128, W * FBLK], bf16)
            # zts with persistent ones row (bias via [W; b] matmul)
            ztsA = cpool.tile([D + 1, 128], f32)
            ztsB = cpool.tile([D + 1, 128], f32)
            zts2 = [ztsA, ztsB]
            for z in zts2:
                nc.vector.memset(z[D:D + 1, :], 1.0)

            rdests = [(0, k) for k in range(W)]

            def maybe_critical():
                # criticals only guard cross-core sem waits, which the
                # timing build strips; skip their all-engine barriers there
                if _TIMING:
                    return contextlib.nullcontext()
                return tc.tile_critical()

            # boundary piece split: A = supertiles 0..NSUP-2 (queue 0),
            # B = last supertile (queue 1). A is triggered as soon as its
            # epilogue data exists, so its exchange + table copy overlap
            # the last supertile's compute.
            AE = (NSUP - 1) * SUP * D

            def bcast_piece(cccur, lo, sz, rs):
                if _TIMING:
                    prep = nc.gpsimd.remote_dma_broadcast(
                        tabsb[:, lo:lo + sz], cccur[:, lo:lo + sz],
                        remote_sem=rs, local_sem=lsem, rdests=rdests)
                    prep.then_inc(psem, 1)
                else:
                    rank = nc.gpsimd.partition_id()
                    for j in nc.gpsimd.Switch(rank, W):
                        prep = nc.gpsimd.remote_dma_broadcast(
                            tabsb[:, j * FBLK + lo:j * FBLK + lo + sz],
                            cccur[:, lo:lo + sz],
                            remote_sem=rs, local_sem=lsem, rdests=rdests)
                        prep.then_inc(psem, 1)
                pcnt[0] += 1
                nc.gpsimd.wait_ge(psem, pcnt[0])
                nc.gpsimd.trigger_dma(count=1)

            def copy_piece(b, lo, sz, rs):
                dst = cc_t[b][:, :].rearrange(
                    "(p a) c -> p (a c)", p=128).rearrange(
                    "p (c f) -> p c f", c=W)[:, :, lo:lo + sz]
                src = tabsb[:].rearrange(
                    "p (c f) -> p c f", c=W)[:, :, lo:lo + sz]
                cpy = nc.sync.dma_start(dst, src)
                if not _TIMING:
                    cpy._wait_ge(rs, 16 * (b + 1))
                cpy.then_inc(csem, 16)
                ccnt[0] += 16

            for layer in range(3):
                table = x_table if layer == 0 else cc_t[layer - 1]
                ccprev = (ccA, ccB, ccA)[layer]
                cccur = (ccB, ccA, None)[layer]
                wt = (wat, wbt, wct)[layer]
                DO = D if layer < 2 else DOUT
                zi = [0]

                for g in range(NSUP):
                    cms, thr, sems = {}, {}, {}
                    if layer == 2:
                        obg = stpool.tile([128, SUP, DOUT], f32, tag="obg")
                    last_g = g == NSUP - 1
                    for q in (0, 1):
                        ch = chunks[g * 2 + q]
                        rp = ch["rp"]
                        if rp == 0:
                            continue
                        msg = msgpool.tile([128, P * 128], bf16, tag="msg")
                        k = (layer * 2 * NSUP + g * 2 + q) % 4
                        ib = ch["idx_base"]
                        # first chunk of a layer: small lead gather so the
                        # DMA starts after ~1.3us of desc-gen, not 3.5us
                        splits = ([min(8, rp), rp] if g == 0 and q == 0
                                  else [rp])
                        lo2 = 0
                        for hi2 in splits:
                            if hi2 <= lo2:
                                continue
                            gx = gixt[:, (ib + lo2 * 128) // 16:
                                      (ib + hi2 * 128) // 16]
                            gth = nc.gpsimd.dma_gather(
                                msg[:, lo2 * 128:hi2 * 128].rearrange(
                                    "p (a c) -> p a c", c=128),
                                table[:, :], gx, (hi2 - lo2) * 128,
                                (hi2 - lo2) * 128, 128,
                                single_packet=False)
                            if not _TIMING:
                                gth.then_inc(gsem[k], 16)
                            gcnt[k] += 16
                            lo2 = hi2
                        if not iota_emitted[0]:
                            iota_emitted[0] = True
                            nc.sync.dma_start(sidt[:], sid_in[:])
                            if g0w < max(ntok // 16, 1):
                                nc.sync.dma_start(gixt[:, g0w:],
                                                  gidx_in[:, g0w:])
                            nc.sync.dma_start(ccA[:], x_own[:])
                            nc.sync.dma_start(wat[:], wa[:])
                            nc.sync.dma_start(wbt[:], wb[:])
                            nc.sync.dma_start(wct[:], wc[:])
                            nc.sync.dma_start(dinvt[:], dinv_in[:])
                            nc.vector.tensor_scalar_mul(
                                d02[:], dinvt[:], NEG_SLOPE)
                            nc.vector.tensor_scalar_mul(
                                d08[:], dinvt[:], 1.0 - NEG_SLOPE)
                        oh = ohpool.tile([128, P * 128], bf16, tag="oh")
                        ia = iotat[:]
                        iv = AP(ia.tensor, ia.offset,
                                [list(ia.ap[0]), [P, 128], [1, rp]])
                        sa = sidt[:, (g * 2 + q) * P:(g * 2 + q) * P + rp]
                        sv = AP(sa.tensor, sa.offset,
                                [list(sa.ap[0]), [0, 128], [1, rp]])
                        oa = oh[:]
                        ov = AP(oa.tensor, oa.offset,
                                [list(oa.ap[0]), [P, 128], [1, rp]])
                        nc.vector.tensor_tensor(ov, iv, sv, EQ)
                        cms[q] = (msg, oh, ch)
                        thr[q] = gcnt[k]
                        sems[q] = gsem[k]

                    if layer < 2 and last_g and AE > 0:
                        # boundary piece A: supertiles 0..NSUP-2 are done;
                        # exchange + table copy overlap this supertile
                        with maybe_critical():
                            if not _TIMING and layer > 0:
                                nc.gpsimd.wait_ge(bsem, 16 * layer)
                            bcast_piece(cccur, 0, AE, rsemA)
                            copy_piece(layer, 0, AE, rsemA)

                    # per-tile accumulation groups: tile t's epilogue can
                    # start as soon as its own planes stop, overlapping the
                    # remaining tiles' matmuls
                    pg = psum.tile([128, 512], f32, tag="pg")
                    qwait = set()
                    for trel in range(SUP):
                        t = g * SUP + trel
                        items = [(None, 0)]
                        for q in sorted(cms):
                            items += [(q, a) for a, tr in
                                      enumerate(cms[q][2]["tiles"])
                                      if tr == trel]
                        for j, (q, a) in enumerate(items):
                            st = j == 0
                            sp = j == len(items) - 1
                            if q is None:
                                nc.tensor.matmul(
                                    pg[:, trel * D:trel * D + D],
                                    identb[:], ccprev[:, t * D:t * D + D],
                                    start=st, stop=sp)
                            else:
                                msg, oh, ch = cms[q]
                                oa = oh[:]
                                lw = AP(oa.tensor, oa.offset + a,
                                        [list(oa.ap[0]), [P, 128]])
                                mm = nc.tensor.matmul(
                                    pg[:, trel * D:trel * D + D], lw,
                                    msg[:, a * 128 + q * D:
                                        a * 128 + q * D + D],
                                    start=st, stop=sp)
                                if not _TIMING and q not in qwait:
                                    mm._wait_ge(sems[q], thr[q])
                                    qwait.add(q)

                    # epilogue for this super-tile
                    stage = stpool.tile([128, SUP * D], f32, tag="stage")
                    for trel in range(SUP):
                        t = g * SUP + trel
                        nc.scalar.activation(
                            stage[:, trel * D:trel * D + D],
                            pg[:, trel * D:trel * D + D],
                            ACT.Copy, bias=0.0, scale=dinvt[:, t:t + 1])
                    for trel in range(SUP):
                        t = g * SUP + trel
                        ztp = psum_t.tile([128, 512], f32, tag="ztp")
                        nc.tensor.transpose(
                            ztp[0:D, 0:128],
                            stage[:, trel * D:trel * D + D], ident[:])
                        zz = zts2[zi[0] % 2]
                        zi[0] += 1
                        nc.vector.tensor_copy(zz[0:D, :], ztp[0:D, 0:128])
                        ph = psum_e.tile([128, 512], f32, tag="ph")
                        nc.tensor.matmul(ph[:, 0:DO], zz[:], wt[:],
                                         start=True, stop=True)
                        if layer < 2:
                            # cc = dinv*leaky(ph)
                            #    = (ph*0.2*dinv) + relu(ph*0.8*dinv)
                            rlu = stpool.tile([128, D], f32, tag="rlu")
                            nc.scalar.activation(
                                rlu[:], ph[:, 0:DO], ACT.Relu,
                                bias=0.0, scale=d08[:, t:t + 1])
                            nc.vector.scalar_tensor_tensor(
                                cccur[:, t * D:t * D + D], ph[:, 0:DO],
                                d02[:, t:t + 1], rlu[:],
                                mybir.AluOpType.mult, mybir.AluOpType.add)
                        else:
                            nc.scalar.activation(
                                obg[:, trel, :], ph[:, 0:DO], ACT.Copy)
                            if last_g and trel == SUP - 2 and SUP > 1:
                                nc.sync.dma_start(
                                    outr[g * SUP * 128:
                                         (g * SUP + SUP - 1) * 128, :]
                                    .rearrange("(a p) c -> p a c", p=128),
                                    obg[:, 0:SUP - 1, :])
                    if layer == 2:
                        if last_g and SUP > 1:
                            t0 = g * SUP + SUP - 1
                            nc.sync.dma_start(
                                outr[t0 * 128:(t0 + 1) * 128, :]
                                .rearrange("(a p) c -> p a c", p=128),
                                obg[:, SUP - 1:SUP, :])
                        else:
                            nc.sync.dma_start(
                                outr[g * SUP * 128:(g + 1) * SUP * 128, :]
                                .rearrange("(a p) c -> p a c", p=128),
                                obg[:])

                if layer < 2:
                    b = layer
                    with maybe_critical():
                        if not _TIMING and b > 0 and AE == 0:
                            nc.gpsimd.wait_ge(bsem, 16 * b)
                        bcast_piece(cccur, AE, FBLK - AE, rsemB)
                        copy_piece(b, AE, FBLK - AE, rsemB)
                        nc.gpsimd.wait_ge(csem, ccnt[0])
                        brp = nc.gpsimd.remote_sem_update_broadcast(
                            bsem, lsem2, rdests=rdests)
                        brp.then_inc(psem, 1)
                        pcnt[0] += 1
                        nc.gpsimd.wait_ge(psem, pcnt[0])
                        nc.gpsimd.trigger_dma(count=1)

    nc.compile()
    return nc


_CACHE = {}


def _get_program(key, cfg, edge_index):
    if key in _CACHE:
        return _CACHE[key]
    dinv, perms, sched, gidxs, sids = _preprocess(cfg, edge_index)
    nc = _build(cfg, sched)
    _CACHE[key] = (nc, dinv, perms, sched, gidxs, sids)
    return _CACHE[key]


def kernel(x, edge_index, W0, b0, W1, b1, W2, b2, _cfg=None, _sim=False):
    import ml_dtypes
    x = np.asarray(x, np.float32)
    edge_index = np.asarray(edge_index)
    N, D = x.shape
    DOUT = np.asarray(W2).shape[1]
    cfg = _cfg or _Cfg(N, D, DOUT)
    nc, dinv, perms, sched, gidxs, sids = _get_program(
        (N, edge_index.shape[1]), cfg, edge_index)

    BP, BLK, Wc, NTILES = cfg.BP, cfg.BLK, cfg.W, cfg.NTILES
    P, nchunk = sched['P'], sched['nchunk']

    xs = (x * dinv[:, None]).astype(ml_dtypes.bfloat16)

    def w65(Wm, bv):
        Wm = np.asarray(Wm, np.float32)
        out = np.zeros((Wm.shape[0] + 1, Wm.shape[1]), np.float32)
        out[:-1] = Wm
        out[-1] = np.asarray(bv, np.float32)
        return out

    # full table in (p, c, t) row order (same on every core)
    NT = cfg.NT
    xt = np.zeros((NT, D), ml_dtypes.bfloat16)
    for c in range(Wc):
        lo, hi = c * BLK, min((c + 1) * BLK, N)
        t = perms[c] >> 7
        p = perms[c] & 127
        xt[(p * Wc + c) * NTILES + t] = xs[lo:hi]
    xt = np.ascontiguousarray(xt.reshape(cfg.NPAIR, 128))
    xts = [xt] * Wc

    in_maps = []
    for c in range(Wc):
        lo, hi = c * BLK, min((c + 1) * BLK, N)
        db = np.zeros(BP, np.float32)
        db[perms[c]] = dinv[lo:hi]
        dinv_blk = np.ascontiguousarray(
            db.reshape(NTILES, 128).T).astype(np.float32)
        xo = np.zeros((128, NTILES, D), ml_dtypes.bfloat16)
        t = perms[c] >> 7
        p = perms[c] & 127
        xo[p, t] = xs[lo:hi]
        sid = sids[c].reshape(nchunk * P, 128).T
        im = dict(
            x_table=xts[c],
            x_own=np.ascontiguousarray(xo.reshape(128, NTILES * D)),
            wa=w65(W0, b0), wb=w65(W1, b1), wc=w65(W2, b2),
            dinv_blk=dinv_blk,
            gidx=_wrap16(gidxs[c]) if sched['ntok'] else
            np.zeros((128, 1), np.int16),
            sid=np.ascontiguousarray(sid).astype(ml_dtypes.bfloat16),
        )
        in_maps.append(im)

    if _sim:
        from concourse import bass_interp, libnrt
        libnrt.get_trn2_nc_mapping = lambda: {(0, i): i for i in range(128)}
        libnrt.get_device_id_to_routing_id_mapping = (
            lambda: {d: d for d in range(16)})
        bass_interp.pnc_id_to_device_and_real_nc_index = (
            lambda cid: (0, cid % 8))
        bass_interp.get_device_id_to_routing_id_mapping = lambda: {0: 0}
        bass_interp.nc_to_real_nc = lambda dev, i: i
        sim = bass_interp.MultiCoreSim(nc, Wc)
        for c in range(Wc):
            for k, v in in_maps[c].items():
                sim.cores[c].tensor(k)[:] = v
            sim.cores[c].mem_tensor("outr")[:] = 0
        sim.simulate()
        results = [np.array(sim.cores[c].mem_tensor("outr")).reshape(BP, DOUT)
                   for c in range(Wc)]
    else:
        from concourse.bass_utils import run_bass_kernel_spmd
        res = run_bass_kernel_spmd(nc, in_maps, list(range(Wc)))
        results = [res.results[c]["outr"] for c in range(Wc)]

    out = np.zeros((N, DOUT), np.float32)
    for c in range(Wc):
        lo, hi = c * BLK, min((c + 1) * BLK, N)
        out[lo:hi] = results[c][perms[c]]
    return out


# revision 49
# speedup vs baseline: 1.3907x; 1.3907x over previous
"""3-layer GCN forward on 8 TRN2 NeuronCores (Bass/Tile), v3.

Math: per layer, out = dinv * ((A+I) @ T) @ W + b with T = dinv*h,
dinv = 1/sqrt(deg+1); leaky_relu(0.2) between layers.

v3 changes vs v2:
- Self-loop tokens eliminated: the +T_own term is an identity-matmul into
  the same PSUM accumulation, reading the locally kept own-block tile.
- AllGather replaced by remote_dma_broadcast (SBUF->SBUF to all 8 cores,
  Switch on partition id selects the sender's slice) + one SBUF->DRAM
  copy that rebuilds the gather table; a data-free broadcast barriers
  table_sb reuse.
- Table row order is (p, c, t): node at core c, tile t, slot p lives at
  row (p*8+c)*NTILES+t, so the staging SBUF tile [128, 8*NTILES*64] is
  DRAM-linear and the table copy runs at full descriptor size.
- Gathers are issued per super-tile (SUP tiles x parity) instead of per
  tile: 14 vs 98 SWDGE desc-gen fixed overheads per layer.
- One-hot built in "a-last" layout oh[p, c*P+a] so all tensor_tensor
  operands have stride-1 last dims (DVE 2x mode); matmul lhsT reads
  plane a with a stride-P access pattern.
- Bias rows folded into the weight matmul ([W; b], ones row in lhsT);
  dinv scaling + leaky fused into Activation-engine ops
  (leaky(dinv*x) == dinv*leaky(x) for dinv > 0).
"""
import numpy as np

NEG_SLOPE = 0.2
_TIMING = False  # strip cross-core sem waits so TimelineSim can run
_DEBUG = False


class _Cfg:
    def __init__(self, n_nodes, d_in=64, d_out=4, sup_tiles=None):
        self.W = 8
        self.N = n_nodes
        self.D = d_in
        self.DOUT = d_out
        self.BLK = (n_nodes + self.W - 1) // self.W
        self.BP = ((self.BLK + 127) // 128) * 128
        self.NT = self.W * self.BP
        self.NPAIR = self.NT // 2
        self.NTILES = self.BP // 128
        if sup_tiles is None:
            sup_tiles = 7 if self.NTILES % 7 == 0 else 1
        self.SUP = sup_tiles if self.NTILES % sup_tiles == 0 else 1
        self.NSUP = self.NTILES // self.SUP
        assert self.NPAIR <= 32768, self.NPAIR


def _balance_tiles(cfg, tok_counts):
    """Assign this core's nodes (local ids) to (tile, slot) so that
    per-(tile,parity) token counts are even. tok_counts: [nloc, 2] int.
    Returns perm[nloc] -> tile*128+slot."""
    nloc = tok_counts.shape[0]
    ntiles = cfg.NTILES
    cap = np.full(ntiles, 128, np.int64)
    load = np.zeros((ntiles, 2), np.float64)
    order = np.argsort(-(tok_counts.sum(1)))
    perm = np.zeros(nloc, np.int64)
    slots_used = np.zeros(ntiles, np.int64)
    for n in order:
        t0, t1 = tok_counts[n]
        cand = np.where(slots_used < cap)[0]
        v = np.maximum(load[cand, 0] + t0, load[cand, 1] + t1)
        best = cand[np.argmin(v)]
        perm[n] = best * 128 + slots_used[best]
        slots_used[best] += 1
        load[best, 0] += t0
        load[best, 1] += t1
    return perm


def _refine_caps(cfg, e, perm, caps, iters=4000):
    """Same-parity-tile swap refinement: push per-(tile,parity) loads
    under caps (so plane counts hit floor instead of ceil). Swapping
    nodes between tiles t, t' with t ≡ t' (mod 2) preserves every node's
    source parity ((c+t)&1), so other cores' schedules are unaffected."""
    ntiles = cfg.NTILES
    tile = (perm >> 7).copy()
    slot = (perm & 127).copy()
    loads = np.zeros((ntiles, 2), np.int64)
    np.add.at(loads, (tile, 0), e[:, 0])
    np.add.at(loads, (tile, 1), e[:, 1])
    nodes_in = [np.where(tile == t)[0] for t in range(ntiles)]

    def overs(ld):
        return np.maximum(ld - caps, 0).sum()

    for _ in range(iters):
        over = loads - caps
        om = over.max(1)
        w = int(np.argmax(om))
        if om[w] <= 0:
            break
        q = int(np.argmax(over[w]))
        par_tiles = np.arange(w & 1, ntiles, 2)
        par_tiles = par_tiles[par_tiles != w]
        # most headroom in q first
        t2s = par_tiles[np.argsort(over[par_tiles, q])][:4]
        un = nodes_in[w]
        best = None
        for t2 in t2s:
            vn = nodes_in[t2]
            # delta for swapping u (from w) with v (from t2):
            du = e[un]                     # [nu, 2]
            dv = e[vn]                     # [nv, 2]
            base = (max(over[w, 0], 0) + max(over[w, 1], 0)
                    + max(over[t2, 0], 0) + max(over[t2, 1], 0))
            d = dv[None, :, :] - du[:, None, :]   # [nu, nv, 2] add to w
            nw = loads[w] + d
            nt = loads[t2] - d
            cost = (np.maximum(nw[:, :, 0] - caps[w, 0], 0)
                    + np.maximum(nw[:, :, 1] - caps[w, 1], 0)
                    + np.maximum(nt[:, :, 0] - caps[t2, 0], 0)
                    + np.maximum(nt[:, :, 1] - caps[t2, 1], 0))
            i, j = np.unravel_index(np.argmin(cost), cost.shape)
            gain = base - cost[i, j]
            if best is None or gain > best[0]:
                best = (gain, un[i], vn[j], t2)
        if best is None or best[0] <= 0:
            break
        _, u, v, t2 = best
        loads[w] += e[v] - e[u]
        loads[t2] += e[u] - e[v]
        tile[u], tile[v] = t2, w
        slot[u], slot[v] = slot[v], slot[u]
        nodes_in[w] = np.where(tile == w)[0]
        nodes_in[t2] = np.where(tile == t2)[0]
    return tile * 128 + slot


def _preprocess(cfg, edge_index):
    """Build shared plane schedule + per-core gidx/sid + per-core node
    permutations and dinv. Table row of node (c, t, p) is
    (p*8 + c)*NTILES + t on every core; the boundary exchange picks the
    sender's slice with a Switch on the partition id."""
    W, N, BLK, NTILES = cfg.W, cfg.N, cfg.BLK, cfg.NTILES
    SUP, NSUP = cfg.SUP, cfg.NSUP
    src = np.asarray(edge_index[0], np.int64)
    dst = np.asarray(edge_index[1], np.int64)
    deg = np.bincount(dst, minlength=N).astype(np.float64) + 1.0
    dinv = (1.0 / np.sqrt(deg)).astype(np.float32)

    node_core = np.minimum(np.arange(N) // BLK, W - 1)

    perms = []
    core_edges = []
    for c in range(W):
        lo, hi = c * BLK, min((c + 1) * BLK, N)
        m = (dst >= lo) & (dst < hi)
        s_c, d_c = src[m], dst[m] - lo
        core_edges.append((s_c, d_c))
        nloc = hi - lo
        # proxy parity first; refined with exact rows below
        tc = np.zeros((nloc, 2), np.int64)
        np.add.at(tc, (d_c, s_c % 2), 1)
        perms.append(_balance_tiles(cfg, tc))

    def rows_on(r, nodes):
        """Table row of `nodes` (same on every receiver: slice = sender)."""
        del r
        c = node_core[nodes]
        pos = np.zeros(len(nodes), np.int64)
        for c2 in range(W):
            m2 = c == c2
            if m2.any():
                pos[m2] = perms[c2][nodes[m2] - c2 * BLK]
        t = pos >> 7
        p = pos & 127
        return (p * W + c) * NTILES + t

    for _sweep in range(2):
        for r in range(W):
            s_c, d_c = core_edges[r]
            nloc = min((r + 1) * BLK, N) - r * BLK
            par = rows_on(r, s_c) & 1
            tc = np.zeros((nloc, 2), np.int64)
            np.add.at(tc, (d_c, par), 1)
            perms[r] = _balance_tiles(cfg, tc)

    # cap-targeted refinement: push every (tile,parity) bucket under
    # 8*128 tokens (designated overflow tiles get 9*128) so plane counts
    # hit the floor. Same-parity swaps keep all parities fixed, so the
    # exact counts below stay valid across cores.
    es, totals = [], []
    for r in range(W):
        s_c, d_c = core_edges[r]
        nloc = min((r + 1) * BLK, N) - r * BLK
        par = rows_on(r, s_c) & 1
        tc = np.zeros((nloc, 2), np.int64)
        np.add.at(tc, (d_c, par), 1)
        es.append(tc)
        totals.append(int(tc.sum()))
    capbase = 8 * 128
    ncap = 2 * NTILES * capbase
    OV = 0
    if max(totals) > ncap - 2 * 128:
        OV = int(np.ceil((max(totals) - (ncap - 2 * 128)) / 256.0)) + 1
        OV = min(OV, NTILES)
    caps = np.full((NTILES, 2), capbase, np.int64)
    caps[:OV] += 128
    for r in range(W):
        perms[r] = _refine_caps(cfg, es[r], perms[r], caps)

    per_core_tok = []
    for r in range(W):
        s_c, d_c = core_edges[r]
        rr = rows_on(r, s_c)
        pair, par = rr >> 1, rr & 1
        pos = perms[r][d_c]
        tile, slot = pos >> 7, pos & 127
        per_core_tok.append((pair, par, tile, slot))

    counts = np.zeros((W, NTILES, 2), np.int64)
    for c in range(W):
        _, par, tile, _ = per_core_tok[c]
        np.add.at(counts[c], (tile, par), 1)
    planes_tq = (counts.max(0) + 127) // 128  # [NTILES, 2], may be 0

    # chunks: (g, q); real planes = sum of planes_tq over tiles of g
    chunks = []  # per chunk: dict(g, q, rp, idx_base, tiles=[trel per a])
    idx_base = 0
    for g in range(NSUP):
        for q in (0, 1):
            tl = []
            for trel in range(SUP):
                t = g * SUP + trel
                tl += [trel] * int(planes_tq[t, q])
            chunks.append(dict(g=g, q=q, rp=len(tl), idx_base=idx_base,
                               tiles=tl))
            idx_base += len(tl) * 128
    ntok = idx_base
    P = max((ch["rp"] for ch in chunks), default=1)
    P = max(P, 1)
    nchunk = len(chunks)

    # per-core gidx / sid fill
    gidxs, sids = [], []
    for c in range(W):
        pair, par, tile, slot = per_core_tok[c]
        gi = np.zeros(ntok, np.int64)             # pad -> pair row 0
        sd = np.full((nchunk, P, 128), -1.0, np.float32)
        key = tile * 2 + par
        order = np.argsort(key, kind='stable')
        ks, ps, ss = key[order], pair[order], slot[order]
        bounds = np.searchsorted(ks, np.arange(NTILES * 2 + 1))
        for ci, ch in enumerate(chunks):
            g, q, rp = ch["g"], ch["q"], ch["rp"]
            base = ch["idx_base"]
            a = 0
            for trel in range(SUP):
                t = g * SUP + trel
                u, v = bounds[t * 2 + q], bounds[t * 2 + q + 1]
                cnt = v - u
                npl = int(planes_tq[t, q])
                assert cnt <= npl * 128, (c, t, q, cnt)
                for j in range(npl):
                    lo2 = u + j * 128
                    hi2 = min(u + (j + 1) * 128, v)
                    n2 = max(hi2 - lo2, 0)
                    if n2 > 0:
                        gi[base + a * 128: base + a * 128 + n2] = ps[lo2:hi2]
                        sd[ci, a, :n2] = ss[lo2:hi2]
                    a += 1
            assert a == rp
        gidxs.append(gi.astype(np.int16))
        sids.append(sd)
    sched = dict(ntok=ntok, P=P, nchunk=nchunk, chunks=chunks,
                 planes_tq=planes_tq)
    return dinv, perms, sched, gidxs, sids


def _wrap16(a):
    a = np.asarray(a, np.int16)
    assert a.size % 16 == 0
    w = np.ascontiguousarray(a.reshape(-1, 16).T)
    return np.tile(w, (8, 1))


def _build(cfg, sched):
    import concourse.bacc as bacc
    import concourse.mybir as mybir
    import concourse.tile as tile
    import concourse.masks as masks
    from concourse.bass import AP

    D, DOUT = cfg.D, cfg.DOUT
    W, BP, NPAIR, NTILES = cfg.W, cfg.BP, cfg.NPAIR, cfg.NTILES
    SUP, NSUP = cfg.SUP, cfg.NSUP
    f32, bf16, i16 = mybir.dt.float32, mybir.dt.bfloat16, mybir.dt.int16
    EQ = mybir.AluOpType.is_equal
    ACT = mybir.ActivationFunctionType
    P = sched['P']
    ntok = sched['ntok']
    chunks = sched['chunks']
    FBLK = NTILES * D          # free elems per core block (per partition)

    import contextlib
    nc = bacc.Bacc(None, target_bir_lowering=False,
                   dynamic_dma_scratch_size=65536)
    x_table = nc.dram_tensor("x_table", [NPAIR, 128], bf16,
                             kind="ExternalInput")
    x_own = nc.dram_tensor("x_own", [128, FBLK], bf16, kind="ExternalInput")
    wa = nc.dram_tensor("wa", [D + 1, D], f32, kind="ExternalInput")
    wb = nc.dram_tensor("wb", [D + 1, D], f32, kind="ExternalInput")
    wc = nc.dram_tensor("wc", [D + 1, DOUT], f32, kind="ExternalInput")
    dinv_in = nc.dram_tensor("dinv_blk", [128, NTILES], f32,
                             kind="ExternalInput")
    gidx_in = nc.dram_tensor("gidx", [128, max(ntok // 16, 1)], i16,
                             kind="ExternalInput")
    sid_in = nc.dram_tensor("sid", [128, sched['nchunk'] * P], bf16,
                            kind="ExternalInput")
    outr = nc.dram_tensor("outr", [BP, DOUT], f32, kind="ExternalOutput")
    cc_t = [nc.dram_tensor(f"cc_t{i}", [NPAIR, 128], bf16) for i in (0, 1)]

    with tile.TileContext(nc) as tc:
        with (
            tc.tile_pool(name="const", bufs=1) as cpool,
            tc.tile_pool(name="msg", bufs=2) as msgpool,
            tc.tile_pool(name="oh", bufs=2) as ohpool,
            tc.tile_pool(name="stage", bufs=2) as stpool,
            tc.tile_pool(name="psum", bufs=2, space="PSUM") as psum,
            tc.tile_pool(name="psum_e", bufs=2, space="PSUM") as psum_e,
            tc.tile_pool(name="psum_t", bufs=2, space="PSUM") as psum_t,
        ):
            gsem = [nc.alloc_semaphore(f"gsem{i}") for i in range(4)]
            gcnt = [0] * 4
            psem = nc.alloc_semaphore("psem")
            rsemA = nc.alloc_semaphore("rsemA")
            rsemB = nc.alloc_semaphore("rsemB")
            lsem = nc.alloc_semaphore("lsem")
            lsem2 = nc.alloc_semaphore("lsem2")
            bsem = nc.alloc_semaphore("bsem")
            csem = nc.alloc_semaphore("csem")
            pcnt = [0]
            ccnt = [0]

            wat = cpool.tile([D + 1, D], f32)
            wbt = cpool.tile([D + 1, D], f32)
            wct = cpool.tile([D + 1, DOUT], f32)
            dinvt = cpool.tile([128, NTILES], f32)
            d02 = cpool.tile([128, NTILES], f32)
            d08 = cpool.tile([128, NTILES], f32)
            gixt = cpool.tile([128, max(ntok // 16, 1)], i16)
            # only the first chunk's index slice loads before the first
            # gather; everything else is deferred past its desc-gen
            g0w = min(max(chunks[0]["rp"] * 8, 1), max(ntok // 16, 1))
            nc.sync.dma_start(gixt[:, 0:g0w], gidx_in[:, 0:g0w])
            sidt = cpool.tile([128, sched['nchunk'] * P], bf16)
            ident = cpool.tile([128, 128], f32)
            masks.make_identity(nc, ident[:])
            identb = cpool.tile([128, 128], bf16)
            nc.vector.tensor_copy(identb[:], ident[:])
            # iotaT[p, c*P + a] = c: tiny Pool iota row (0.2us) + DVE
            # broadcast-expand, which runs while DVE is otherwise idle --
            # a full Pool iota would block early gather desc-gen ~10us
            iotar = cpool.tile([128, 128], bf16)
            nc.gpsimd.iota(iotar[:], pattern=[[1, 128]], base=0,
                           channel_multiplier=0,
                           allow_small_or_imprecise_dtypes=True)
            iotat = cpool.tile([128, P * 128], bf16)
            ira = iotar[:]
            nc.vector.tensor_copy(
                AP(iotat[:].tensor, iotat[:].offset,
                   [list(iotat[:].ap[0]), [P, 128], [1, P]]),
                AP(ira.tensor, ira.offset,
                   [list(ira.ap[0]), [1, 128], [0, P]]))
            iota_emitted = [False]
            # own-block staging (prev layer T) + current layer output
            ccA = cpool.tile([128, FBLK], bf16)
            ccB = cpool.tile([128, FBLK], bf16)
            # broadcast assembly + final out buffers
            tabsb = cpool.tile([128, W * FBLK], bf16)
            # zts with persistent ones row (bias via [W; b] matmul)
            ztsA = cpool.tile([D + 1, 128], f32)
            ztsB = cpool.tile([D + 1, 128], f32)
            zts2 = [ztsA, ztsB]
            for z in zts2:
                nc.vector.memset(z[D:D + 1, :], 1.0)

            rdests = [(0, k) for k in range(W)]

            def maybe_critical():
                # criticals only guard cross-core sem waits, which the
                # timing build strips; skip their all-engine barriers there
                if _TIMING:
                    return contextlib.nullcontext()
                return tc.tile_critical()

            # boundary piece split: A = supertiles 0..NSUP-2 (queue 0),
            # B = last supertile (queue 1). A is triggered as soon as its
            # epilogue data exists, so its exchange + table copy overlap
            # the last supertile's compute.
            AE = (NSUP - 1) * SUP * D

            def bcast_piece(cccur, lo, sz, rs):
                if _TIMING:
                    prep = nc.gpsimd.remote_dma_broadcast(
                        tabsb[:, lo:lo + sz], cccur[:, lo:lo + sz],
                        remote_sem=rs, local_sem=lsem, rdests=rdests)
                    prep.then_inc(psem, 1)
                else:
                    rank = nc.gpsimd.partition_id()
                    for j in nc.gpsimd.Switch(rank, W):
                        prep = nc.gpsimd.remote_dma_broadcast(
                            tabsb[:, j * FBLK + lo:j * FBLK + lo + sz],
                            cccur[:, lo:lo + sz],
                            remote_sem=rs, local_sem=lsem, rdests=rdests)
                        prep.then_inc(psem, 1)
                pcnt[0] += 1
                nc.gpsimd.wait_ge(psem, pcnt[0])
                nc.gpsimd.trigger_dma(count=1)

            def copy_piece(b, lo, sz, rs):
                dst = cc_t[b][:, :].rearrange(
                    "(p a) c -> p (a c)", p=128).rearrange(
                    "p (c f) -> p c f", c=W)[:, :, lo:lo + sz]
                src = tabsb[:].rearrange(
                    "p (c f) -> p c f", c=W)[:, :, lo:lo + sz]
                cpy = nc.sync.dma_start(dst, src)
                if not _TIMING:
                    cpy._wait_ge(rs, 16 * (b + 1))
                cpy.then_inc(csem, 16)
                ccnt[0] += 16

            for layer in range(3):
                table = x_table if layer == 0 else cc_t[layer - 1]
                ccprev = (ccA, ccB, ccA)[layer]
                cccur = (ccB, ccA, None)[layer]
                wt = (wat, wbt, wct)[layer]
                DO = D if layer < 2 else DOUT
                zi = [0]

                for g in range(NSUP):
                    cms, thr, sems = {}, {}, {}
                    if layer == 2:
                        obg = stpool.tile([128, SUP, DOUT], f32, tag="obg")
                    last_g = g == NSUP - 1
                    for q in (0, 1):
                        ch = chunks[g * 2 + q]
                        rp = ch["rp"]
                        if rp == 0:
                            continue
                        msg = msgpool.tile([128, P * 128], bf16, tag="msg")
                        k = (layer * 2 * NSUP + g * 2 + q) % 4
                        ib = ch["idx_base"]
                        # first chunk of a layer: small lead gather so the
                        # DMA starts after ~1.3us of desc-gen, not 3.5us
                        splits = ([min(8, rp), rp] if g == 0 and q == 0
                                  else [rp])
                        lo2 = 0
                        for hi2 in splits:
                            if hi2 <= lo2:
                                continue
                            gx = gixt[:, (ib + lo2 * 128) // 16:
                                      (ib + hi2 * 128) // 16]
                            gth = nc.gpsimd.dma_gather(
                                msg[:, lo2 * 128:hi2 * 128].rearrange(
                                    "p (a c) -> p a c", c=128),
                                table[:, :], gx, (hi2 - lo2) * 128,
                                (hi2 - lo2) * 128, 128,
                                single_packet=False)
                            if not _TIMING:
                                gth.then_inc(gsem[k], 16)
                            gcnt[k] += 16
                            lo2 = hi2
                        if not iota_emitted[0]:
                            iota_emitted[0] = True
                            nc.sync.dma_start(sidt[:], sid_in[:])
                            if g0w < max(ntok // 16, 1):
                                nc.sync.dma_start(gixt[:, g0w:],
                                                  gidx_in[:, g0w:])
                            nc.sync.dma_start(ccA[:], x_own[:])
                            nc.sync.dma_start(wat[:], wa[:])
                            nc.sync.dma_start(wbt[:], wb[:])
                            nc.sync.dma_start(wct[:], wc[:])
                            nc.sync.dma_start(dinvt[:], dinv_in[:])
                            nc.vector.tensor_scalar_mul(
                                d02[:], dinvt[:], NEG_SLOPE)
                            nc.vector.tensor_scalar_mul(
                                d08[:], dinvt[:], 1.0 - NEG_SLOPE)
                        oh = ohpool.tile([128, P * 128], bf16, tag="oh")
                        ia = iotat[:]
                        iv = AP(ia.tensor, ia.offset,
                                [list(ia.ap[0]), [P, 128], [1, rp]])
                        sa = sidt[:, (g * 2 + q) * P:(g * 2 + q) * P + rp]
                        sv = AP(sa.tensor, sa.offset,
                                [list(sa.ap[0]), [0, 128], [1, rp]])
                        oa = oh[:]
                        ov = AP(oa.tensor, oa.offset,
                                [list(oa.ap[0]), [P, 128], [1, rp]])
                        nc.vector.tensor_tensor(ov, iv, sv, EQ)
                        cms[q] = (msg, oh, ch)
                        thr[q] = gcnt[k]
                        sems[q] = gsem[k]

                    if layer < 2 and last_g and AE > 0:
                        # boundary piece A: supertiles 0..NSUP-2 are done;
                        # exchange + table copy overlap this supertile
                        with maybe_critical():
                            if not _TIMING and layer > 0:
                                nc.gpsimd.wait_ge(bsem, 16 * layer)
                            bcast_piece(cccur, 0, AE, rsemA)
                            copy_piece(layer, 0, AE, rsemA)

                    pg = psum.tile([128, 512], f32, tag="pg")
                    nmm = sum(cms[q][2]["rp"] for q in cms) + SUP
                    i = 0
                    for q in sorted(cms):
                        msg, oh, ch = cms[q]
                        oa = oh[:]
                        first = True
                        for a, trel in enumerate(ch["tiles"]):
                            lw = AP(oa.tensor, oa.offset + a,
                                    [list(oa.ap[0]), [P, 128]])
                            mm = nc.tensor.matmul(
                                pg[:, trel * D:trel * D + D],
                                lw,
                                msg[:, a * 128 + q * D:a * 128 + q * D + D],
                                start=(i == 0), stop=False)
                            if not _TIMING and first:
                                mm._wait_ge(sems[q], thr[q])
                                first = False
                            i += 1
                    for trel in range(SUP):
                        t = g * SUP + trel
                        nc.tensor.matmul(
                            pg[:, trel * D:trel * D + D], identb[:],
                            ccprev[:, t * D:t * D + D],
                            start=(i == 0), stop=(i == nmm - 1))
                        i += 1

                    # epilogue for this super-tile
                    stage = stpool.tile([128, SUP * D], f32, tag="stage")
                    for trel in range(SUP):
                        t = g * SUP + trel
                        nc.scalar.activation(
                            stage[:, trel * D:trel * D + D],
                            pg[:, trel * D:trel * D + D],
                            ACT.Copy, bias=0.0, scale=dinvt[:, t:t + 1])
                    for trel in range(SUP):
                        t = g * SUP + trel
                        ztp = psum_t.tile([128, 512], f32, tag="ztp")
                        nc.tensor.transpose(
                            ztp[0:D, 0:128],
                            stage[:, trel * D:trel * D + D], ident[:])
                        zz = zts2[zi[0] % 2]
                        zi[0] += 1
                        nc.vector.tensor_copy(zz[0:D, :], ztp[0:D, 0:128])
                        ph = psum_e.tile([128, 512], f32, tag="ph")
                        nc.tensor.matmul(ph[:, 0:DO], zz[:], wt[:],
                                         start=True, stop=True)
                        if layer < 2:
                            # cc = dinv*leaky(ph)
                            #    = (ph*0.2*dinv) + relu(ph*0.8*dinv)
                            rlu = stpool.tile([128, D], f32, tag="rlu")
                            nc.scalar.activation(
                                rlu[:], ph[:, 0:DO], ACT.Relu,
                                bias=0.0, scale=d08[:, t:t + 1])
                            nc.vector.scalar_tensor_tensor(
                                cccur[:, t * D:t * D + D], ph[:, 0:DO],
                                d02[:, t:t + 1], rlu[:],
                                mybir.AluOpType.mult, mybir.AluOpType.add)
                        else:
                            nc.scalar.activation(
                                obg[:, trel, :], ph[:, 0:DO], ACT.Copy)
                            if last_g and trel == SUP - 2 and SUP > 1:
                                nc.sync.dma_start(
                                    outr[g * SUP * 128:
                                         (g * SUP + SUP - 1) * 128, :]
                                    .rearrange("(a p) c -> p a c", p=128),
                                    obg[:, 0:SUP - 1, :])
                    if layer == 2:
                        if last_g and SUP > 1:
                            t0 = g * SUP + SUP - 1
                            nc.sync.dma_start(
                                outr[t0 * 128:(t0 + 1) * 128, :]
                                .rearrange("(a p) c -> p a c", p=128),
                                obg[:, SUP - 1:SUP, :])
                        else:
                            nc.sync.dma_start(
                                outr[g * SUP * 128:(g + 1) * SUP * 128, :]
                                .rearrange("(a p) c -> p a c", p=128),
                                obg[:])

                if layer < 2:
                    b = layer
                    with maybe_critical():
                        if not _TIMING and b > 0 and AE == 0:
                            nc.gpsimd.wait_ge(bsem, 16 * b)
                        bcast_piece(cccur, AE, FBLK - AE, rsemB)
                        copy_piece(b, AE, FBLK - AE, rsemB)
                        nc.gpsimd.wait_ge(csem, ccnt[0])
                        brp = nc.gpsimd.remote_sem_update_broadcast(
                            bsem, lsem2, rdests=rdests)
                        brp.then_inc(psem, 1)
                        pcnt[0] += 1
                        nc.gpsimd.wait_ge(psem, pcnt[0])
                        nc.gpsimd.trigger_dma(count=1)

    nc.compile()
    return nc


_CACHE = {}


def _get_program(key, cfg, edge_index):
    if key in _CACHE:
        return _CACHE[key]
    dinv, perms, sched, gidxs, sids = _preprocess(cfg, edge_index)
    nc = _build(cfg, sched)
    _CACHE[key] = (nc, dinv, perms, sched, gidxs, sids)
    return _CACHE[key]


def kernel(x, edge_index, W0, b0, W1, b1, W2, b2, _cfg=None, _sim=False):
    import ml_dtypes
    x = np.asarray(x, np.float32)
    edge_index = np.asarray(edge_index)
    N, D = x.shape
    DOUT = np.asarray(W2).shape[1]
    cfg = _cfg or _Cfg(N, D, DOUT)
    nc, dinv, perms, sched, gidxs, sids = _get_program(
        (N, edge_index.shape[1]), cfg, edge_index)

    BP, BLK, Wc, NTILES = cfg.BP, cfg.BLK, cfg.W, cfg.NTILES
    P, nchunk = sched['P'], sched['nchunk']

    xs = (x * dinv[:, None]).astype(ml_dtypes.bfloat16)

    def w65(Wm, bv):
        Wm = np.asarray(Wm, np.float32)
        out = np.zeros((Wm.shape[0] + 1, Wm.shape[1]), np.float32)
        out[:-1] = Wm
        out[-1] = np.asarray(bv, np.float32)
        return out

    # full table in (p, c, t) row order (same on every core)
    NT = cfg.NT
    xt = np.zeros((NT, D), ml_dtypes.bfloat16)
    for c in range(Wc):
        lo, hi = c * BLK, min((c + 1) * BLK, N)
        t = perms[c] >> 7
        p = perms[c] & 127
        xt[(p * Wc + c) * NTILES + t] = xs[lo:hi]
    xt = np.ascontiguousarray(xt.reshape(cfg.NPAIR, 128))
    xts = [xt] * Wc

    in_maps = []
    for c in range(Wc):
        lo, hi = c * BLK, min((c + 1) * BLK, N)
        db = np.zeros(BP, np.float32)
        db[perms[c]] = dinv[lo:hi]
        dinv_blk = np.ascontiguousarray(
            db.reshape(NTILES, 128).T).astype(np.float32)
        xo = np.zeros((128, NTILES, D), ml_dtypes.bfloat16)
        t = perms[c] >> 7
        p = perms[c] & 127
        xo[p, t] = xs[lo:hi]
        sid = sids[c].reshape(nchunk * P, 128).T
        im = dict(
            x_table=xts[c],
            x_own=np.ascontiguousarray(xo.reshape(128, NTILES * D)),
            wa=w65(W0, b0), wb=w65(W1, b1), wc=w65(W2, b2),
            dinv_blk=dinv_blk,
            gidx=_wrap16(gidxs[c]) if sched['ntok'] else
            np.zeros((128, 1), np.int16),
            sid=np.ascontiguousarray(sid).astype(ml_dtypes.bfloat16),
        )
        in_maps.append(im)

    if _sim:
        from concourse import bass_interp, libnrt
        libnrt.get_trn2_nc_mapping = lambda: {(0, i): i for i in range(128)}
        libnrt.get_device_id_to_routing_id_mapping = (
            lambda: {d: d for d in range(16)})
        bass_interp.pnc_id_to_device_and_real_nc_index = (
            lambda cid: (0, cid % 8))
        bass_interp.get_device_id_to_routing_id_mapping = lambda: {0: 0}
        bass_interp.nc_to_real_nc = lambda dev, i: i
        sim = bass_interp.MultiCoreSim(nc, Wc)
        for c in range(Wc):
            for k, v in in_maps[c].items():
                sim.cores[c].tensor(k)[:] = v
            sim.cores[c].mem_tensor("outr")[:] = 0
        sim.simulate()
        results = [np.array(sim.cores[c].mem_tensor("outr")).reshape(BP, DOUT)
                   for c in range(Wc)]
    else:
        from concourse.bass_utils import run_bass_kernel_spmd
        res = run_bass_kernel_spmd(nc, in_maps, list(range(Wc)))
        results = [res.results[c]["outr"] for c in range(Wc)]

    out = np.zeros((N, DOUT), np.float32)
    for c in range(Wc):
        lo, hi = c * BLK, min((c + 1) * BLK, N)
        out[lo:hi] = results[c][perms[c]]
    return out
